# revision 1
# baseline (speedup 1.0000x reference)
"""Trainium2 Bass kernel for the contrastive loss (nn_Contrast).

loss = LAM * mean_i(-log s_mp[i]) + (1-LAM) * mean_i(-log s_sc[i])
  S = exp(cos(n1_i, n2_j)/tau);  n1 = norm(proj(z_mp)), n2 = norm(proj(z_sc))
  s_mp[i] = sum_j S[i, c_ij] / rowsum_i ;  s_sc[i] = sum_j S[c_ij, i] / colsum_i

Sharding: rows of S across 8 cores (1024 rows each). Each core:
  - projects its z_mp row-block (transposed pipeline, bf16 matmuls)
  - projects the FULL z_sc (redundant; needed as rhs of its S row-block)
  - streams its S block tile-by-tile: exp (with per-row 1/(norm*tau) scale
    folded into the ACT scale), rowsum via ACT accum, colsum + masked
    column-sums via PE ones-matmuls, mp-edge extraction via masked
    tensor_tensor_reduce. Edge masks are built host-side from pos.
  - one 64KB AllReduce combines colsum and the sc-edge numerator partials.
Host combines 8 partial scalars.
"""

import numpy as np
import ml_dtypes

N = 8192
HID = 512
TAU = 0.8
LAM = 0.5
NCORES = 8
B = N // NCORES          # rows per core = 1024
RT = B // 128            # row tiles per core = 8
CC = N // 1024           # 1024-wide col chunks = 8
KT = HID // 128          # contraction tiles = 4

bf16 = ml_dtypes.bfloat16


def _split_multi_waits(nc, mybir):
    """This container's walrus accepts only ONE sync-wait per instruction;
    Tile batches several. Split extras into single-wait NoOps."""
    counter = [0]
    for f in nc.m.functions:
        for bb in f.blocks:
            new_insts = []
            changed = False
            for inst in bb.instructions:
                si = inst.sync_info
                if si is not None and si.on_wait is not None and len(si.on_wait) > 1:
                    waits = list(si.on_wait)
                    for w in waits[:-1]:
                        counter[0] += 1
                        new_insts.append(mybir.InstNoOp(
                            name=f"I-wsplit-{counter[0]}",
                            engine=inst.engine,
                            sync_info=mybir.SyncInfo(on_wait=[w], on_update=[]),
                            bass_nofuse=True,
                        ))
                    inst.sync_info = mybir.SyncInfo(
                        on_wait=[waits[-1]], on_update=list(si.on_update or []))
                    changed = True
                new_insts.append(inst)
            if changed:
                bb.instructions = new_insts
    return nc


def build_program():
    import concourse.bass as bass
    import concourse.mybir as mybir
    import concourse.tile as tile

    dt = mybir.dt
    F32, BF16 = dt.float32, dt.bfloat16
    Act = mybir.ActivationFunctionType
    Alu = mybir.AluOpType

    nc = bass.Bass("TRN2", num_devices=NCORES)

    z_mpt = nc.dram_tensor("z_mpt", [HID, B], BF16, kind="ExternalInput")
    z_sct = nc.dram_tensor("z_sct", [HID, N], BF16, kind="ExternalInput")
    w1t = nc.dram_tensor("w1t", [HID, HID], BF16, kind="ExternalInput")
    w2t = nc.dram_tensor("w2t", [HID, HID], BF16, kind="ExternalInput")
    b1r = nc.dram_tensor("b1r", [1, HID], BF16, kind="ExternalInput")
    b2r = nc.dram_tensor("b2r", [1, HID], BF16, kind="ExternalInput")
    mask_mp = nc.dram_tensor("mask_mp", [CC, RT, 128, 1024], BF16,
                             kind="ExternalInput")
    mask_sc = nc.dram_tensor("mask_sc", [CC, RT, 128, 1024], BF16,
                             kind="ExternalInput")
    out = nc.dram_tensor("out", [1, 2], F32, kind="ExternalOutput")

    rn1_dram = nc.dram_tensor("rn1_dram", [B], F32)
    norm_dram = nc.dram_tensor("norm_dram", [N], F32)
    rn_dram = nc.dram_tensor("rn_dram", [N], F32)
    cc_in = nc.dram_tensor("cc_in", [2, N], F32)
    cc_out = nc.dram_tensor("cc_out", [2, N], F32, addr_space="Shared")

    with tile.TileContext(nc) as tc:
        with tc.tile_pool(name="const", bufs=1) as constp, \
             tc.tile_pool(name="persist", bufs=1) as pers:
            ones_row = constp.tile([1, 1024], BF16, tag="ones_row", name="ones_row")
            nc.vector.memset(ones_row[:], 1.0)
            ones_row_f32 = constp.tile([1, 128], F32, tag="ones_row_f32", name="ones_row_f32")
            nc.vector.memset(ones_row_f32[:], 1.0)
            ones_col = constp.tile([128, 1], BF16, tag="ones_col", name="ones_col")
            nc.vector.memset(ones_col[:], 1.0)
            ones_col_f32 = constp.tile([128, 1], F32, tag="ones_col_f32", name="ones_col_f32")
            nc.vector.memset(ones_col_f32[:], 1.0)

            w1s = [constp.tile([128, HID], BF16, tag=f"w1_{k}", name=f"w1_{k}") for k in range(KT)]
            w2s = [constp.tile([128, HID], BF16, tag=f"w2_{k}", name=f"w2_{k}") for k in range(KT)]
            for k in range(KT):
                nc.sync.dma_start(out=w1s[k][:], in_=w1t[k * 128:(k + 1) * 128, :])
                nc.sync.dma_start(out=w2s[k][:], in_=w2t[k * 128:(k + 1) * 128, :])
            b1s = constp.tile([1, HID], BF16, tag="b1s", name="b1s")
            nc.sync.dma_start(out=b1s[:], in_=b1r[:])
            b2s = constp.tile([1, HID], BF16, tag="b2s", name="b2s")
            nc.sync.dma_start(out=b2s[:], in_=b2r[:])

            # persistent results
            p1T = [pers.tile([128, B], BF16, tag=f"p1T_{k}", name=f"p1T_{k}") for k in range(KT)]
            n2T = [pers.tile([128, N], BF16, tag=f"n2T_{k}", name=f"n2T_{k}") for k in range(KT)]
            scale_mp = pers.tile([128, RT], F32, tag="scale_mp", name="scale_mp")
            rowsum_acc = pers.tile([128, RT * CC], F32, tag="rowsum_acc", name="rowsum_acc")
            nummp_acc = pers.tile([128, RT * CC], F32, tag="nummp_acc", name="nummp_acc")
            out_sb = pers.tile([1, 2], F32, tag="out_sb", name="out_sb")

            # ---------------- Stage A: proj(z_mp block) -> p1T, scale_mp
            with tc.tile_pool(name="stA", bufs=1) as stA, \
                 tc.tile_pool(name="workA", bufs=2) as wkA, \
                 tc.tile_pool(name="psA", bufs=2, space="PSUM") as psA, \
                 tc.tile_pool(name="psA1", bufs=1, space="PSUM") as psA1:
                zmp = [stA.tile([128, B], BF16, tag=f"zmp_{k}", name=f"zmp_{k}") for k in range(KT)]
                for k in range(KT):
                    nc.sync.dma_start(out=zmp[k][:],
                                      in_=z_mpt[k * 128:(k + 1) * 128, :])
                h1 = [stA.tile([128, B], BF16, tag=f"h1_{k}", name=f"h1_{k}") for k in range(KT)]
                for ht in range(KT):
                    hsl = slice(ht * 128, (ht + 1) * 128)
                    ps = psA.tile([128, B], F32, tag="psA", name="psA")
                    for h in range(B // 512):
                        sl = slice(h * 512, (h + 1) * 512)
                        for k in range(KT):
                            nc.tensor.matmul(ps[:, sl], w1s[k][:, hsl],
                                             zmp[k][:, sl],
                                             start=(k == 0), stop=False)
                        nc.tensor.matmul(ps[:, sl], b1s[0:1, hsl],
                                         ones_row[0:1, 0:512],
                                         start=False, stop=True)
                    tmin = wkA.tile([128, B], BF16, tag="tmin", name="tmin")
                    nc.vector.tensor_scalar_min(tmin[:], ps[:], 0.0)
                    texp = wkA.tile([128, B], BF16, tag="texp", name="texp")
                    nc.scalar.activation(texp[:], tmin[:], Act.Exp)
                    nc.vector.scalar_tensor_tensor(h1[ht][:], texp[:], -1.0, ps[:],
                                                   op0=Alu.add, op1=Alu.max)
                norm2h = [psA1.tile([1, 512], F32, tag=f"norm2A_{h}", name=f"norm2A_{h}")
                          for h in range(B // 512)]
                for ht in range(KT):
                    hsl = slice(ht * 128, (ht + 1) * 128)
                    ps2 = psA.tile([128, B], F32, tag="psA", name="psA2")
                    for h in range(B // 512):
                        sl = slice(h * 512, (h + 1) * 512)
                        for k in range(KT):
                            nc.tensor.matmul(ps2[:, sl], w2s[k][:, hsl],
                                             h1[k][:, sl],
                                             start=(k == 0), stop=False)
                        nc.tensor.matmul(ps2[:, sl], b2s[0:1, hsl],
                                         ones_row[0:1, 0:512],
                                         start=False, stop=True)
                    sq = wkA.tile([128, B], BF16, tag="sqA", name="sqA")
                    nc.scalar.activation(sq[:], ps2[:], Act.Square)
                    for h in range(B // 512):
                        sl = slice(h * 512, (h + 1) * 512)
                        nc.tensor.matmul(norm2h[h][0:1, :], ones_col[:], sq[:, sl],
                                         start=(ht == 0), stop=(ht == KT - 1))
                    nc.vector.tensor_copy(p1T[ht][:], ps2[:])
                nrm = wkA.tile([1, B], F32, tag="nrmA", name="nrmA")
                for h in range(B // 512):
                    sl = slice(h * 512, (h + 1) * 512)
                    nc.scalar.activation(nrm[0:1, sl], norm2h[h][:], Act.Sqrt)
                rn1 = wkA.tile([1, B], F32, tag="rn1A", name="rn1A")
                nc.vector.reciprocal(rn1[:], nrm[:])
                nc.vector.tensor_scalar_mul(rn1[:], rn1[:], 1.0 / TAU)
                nc.gpsimd.dma_start(out=rn1_dram[:], in_=rn1[:])
                nc.gpsimd.dma_start(
                    out=scale_mp[:],
                    in_=rn1_dram[:].rearrange("(g p) -> p g", p=128))

            # ---------------- Stage B: proj(full z_sc) -> n2T (normalized)
            with tc.tile_pool(name="h2p", bufs=1) as h2p:
                with tc.tile_pool(name="zscp", bufs=2) as zscp, \
                     tc.tile_pool(name="psB", bufs=2, space="PSUM") as psB, \
                     tc.tile_pool(name="workB", bufs=2) as wkB:
                    h2 = [h2p.tile([128, N], BF16, tag=f"h2_{k}", name=f"h2_{k}")
                          for k in range(KT)]
                    for nch in range(N // 1024):
                        nsl = slice(nch * 1024, (nch + 1) * 1024)
                        zc = [zscp.tile([128, 1024], BF16, tag=f"zc_{k}", name=f"zc_{k}")
                              for k in range(KT)]
                        for k in range(KT):
                            nc.sync.dma_start(out=zc[k][:],
                                              in_=z_sct[k * 128:(k + 1) * 128, nsl])
                        for ht in range(KT):
                            hsl = slice(ht * 128, (ht + 1) * 128)
                            ps = psB.tile([128, 1024], F32, tag="psB", name="psB")
                            for h in range(2):
                                psl = slice(h * 512, (h + 1) * 512)
                                for k in range(KT):
                                    nc.tensor.matmul(ps[:, psl], w1s[k][:, hsl],
                                                     zc[k][:, psl],
                                                     start=(k == 0), stop=False)
                                nc.tensor.matmul(ps[:, psl], b1s[0:1, hsl],
                                                 ones_row[0:1, 0:512],
                                                 start=False, stop=True)
                            tmin = wkB.tile([128, 1024], BF16, tag="tminB", name="tminB")
                            nc.vector.tensor_scalar_min(tmin[:], ps[:], 0.0)
                            texp = wkB.tile([128, 1024], BF16, tag="texpB", name="texpB")
                            nc.scalar.activation(texp[:], tmin[:], Act.Exp)
                            nc.vector.scalar_tensor_tensor(
                                h2[ht][:, nsl], texp[:], -1.0, ps[:],
                                op0=Alu.add, op1=Alu.max)

                # layer 2: unscaled p2T -> n2T tiles; norms accumulated to DRAM
                with tc.tile_pool(name="psB2", bufs=1, space="PSUM") as psB2, \
                     tc.tile_pool(name="psB2n", bufs=2, space="PSUM") as psB2n, \
                     tc.tile_pool(name="workB2", bufs=3) as wkB2:
                    for nch in range(N // 512):
                        nsl = slice(nch * 512, (nch + 1) * 512)
                        pst = [psB2.tile([128, 512], F32, tag=f"pstB2_{ht}", name=f"pstB2_{ht}")
                               for ht in range(KT)]
                        norm2 = psB2n.tile([1, 512], F32, tag="norm2B", name="norm2B")
                        for ht in range(KT):
                            hsl = slice(ht * 128, (ht + 1) * 128)
                            for k in range(KT):
                                nc.tensor.matmul(pst[ht][:], w2s[k][:, hsl],
                                                 h2[k][:, nsl],
                                                 start=(k == 0), stop=False)
                            nc.tensor.matmul(pst[ht][:], b2s[0:1, hsl],
                                             ones_row[0:1, 0:512],
                                             start=False, stop=True)
                            sq = wkB2.tile([128, 512], BF16, tag="sqB", name="sqB")
                            nc.scalar.activation(sq[:], pst[ht][:], Act.Square)
                            nc.tensor.matmul(norm2[0:1, :], ones_col[:], sq[:],
                                             start=(ht == 0), stop=(ht == KT - 1))
                            nc.vector.tensor_copy(n2T[ht][:, nsl], pst[ht][:])
                        nb2 = wkB2.tile([1, 512], F32, tag="nb2", name="nb2")
                        nc.scalar.copy(nb2[:], norm2[:])
                        nc.sync.dma_start(out=norm_dram[nch * 512:(nch + 1) * 512],
                                          in_=nb2[:])
                    # batch rsqrt in [128, 64] layout, back to a row via DRAM
                    nt = wkB2.tile([128, 64], F32, tag="ntB", name="ntB")
                    nc.sync.dma_start(
                        out=nt[:], in_=norm_dram[:].rearrange("(p f) -> p f", p=128))
                    nrt_ = wkB2.tile([128, 64], F32, tag="nrtB", name="nrtB")
                    nc.scalar.activation(nrt_[:], nt[:], Act.Sqrt)
                    rnt = wkB2.tile([128, 64], F32, tag="rntB", name="rntB")
                    nc.vector.reciprocal(rnt[:], nrt_[:])
                    nc.sync.dma_start(out=rn_dram[:].rearrange("(p f) -> p f", p=128),
                                      in_=rnt[:])
                    # scale n2T columns in place, 512 at a time
                    for nch in range(N // 512):
                        nsl = slice(nch * 512, (nch + 1) * 512)
                        rn2 = wkB2.tile([1, 512], F32, tag="rn2B", name="rn2B")
                        nc.sync.dma_start(out=rn2[:],
                                          in_=rn_dram[nch * 512:(nch + 1) * 512])
                        repl = psB2n.tile([128, 512], F32, tag="replB", name="replB")
                        nc.tensor.matmul(repl[:], ones_row_f32[:], rn2[:],
                                         start=True, stop=True)
                        repl_sb = wkB2.tile([128, 512], BF16, tag="replsbB", name="replsbB")
                        nc.scalar.copy(repl_sb[:], repl[:])
                        for ht in range(KT):
                            nc.vector.tensor_tensor(n2T[ht][:, nsl], n2T[ht][:, nsl],
                                                    repl_sb[:], op=Alu.mult)

            # ---------------- Stage C: S block sweep
            with tc.tile_pool(name="workC", bufs=3) as wkC, \
                 tc.tile_pool(name="maskC", bufs=3) as mkC, \
                 tc.tile_pool(name="psC", bufs=2, space="PSUM") as psC, \
                 tc.tile_pool(name="psCa", bufs=1, space="PSUM") as psCa:
                for cc in range(CC):
                    csum = [psCa.tile([1, 512], F32, tag=f"csum_{h}", name=f"csum_{h}")
                            for h in range(2)]
                    nsum = [psCa.tile([1, 512], F32, tag=f"nsum_{h}", name=f"nsum_{h}")
                            for h in range(2)]
                    for rt in range(RT):
                        rsl = slice(rt * 128, (rt + 1) * 128)
                        sp = psC.tile([128, 1024], F32, tag="spC", name="spC")
                        for k in range(KT):
                            for h in range(2):
                                sl = slice(cc * 1024 + h * 512,
                                           cc * 1024 + (h + 1) * 512)
                                psl = slice(h * 512, (h + 1) * 512)
                                nc.tensor.matmul(sp[:, psl], p1T[k][:, rsl],
                                                 n2T[k][:, sl],
                                                 start=(k == 0),
                                                 stop=(k == KT - 1))
                        s_sb = wkC.tile([128, 1024], BF16, tag="s_sb", name="s_sb")
                        idx = rt * CC + cc
                        nc.scalar.activation(s_sb[:], sp[:], Act.Exp,
                                             scale=scale_mp[:, rt:rt + 1],
                                             accum_out=rowsum_acc[:, idx:idx + 1])
                        mmp = mkC.tile([128, 1024], BF16, tag="mmp", name="mmp")
                        nc.sync.dma_start(out=mmp[:], in_=mask_mp[cc, rt])
                        msc = mkC.tile([128, 1024], BF16, tag="msc", name="msc")
                        nc.sync.dma_start(out=msc[:], in_=mask_sc[cc, rt])
                        ttro = wkC.tile([128, 1024], BF16, tag="ttro", name="ttro")
                        nc.vector.scalar_tensor_tensor(
                            ttro[:], s_sb[:], 1.0, mmp[:],
                            op0=Alu.mult, op1=Alu.mult,
                            accum_out=nummp_acc[:, idx:idx + 1])
                        msk = wkC.tile([128, 1024], BF16, tag="msk", name="msk")
                        nc.vector.tensor_tensor(msk[:], s_sb[:], msc[:],
                                                op=Alu.mult)
                        for h in range(2):
                            psl = slice(h * 512, (h + 1) * 512)
                            nc.tensor.matmul(csum[h][0:1, :], ones_col[:],
                                             s_sb[:, psl],
                                             start=(rt == 0), stop=(rt == RT - 1))
                            nc.tensor.matmul(nsum[h][0:1, :], ones_col[:],
                                             msk[:, psl],
                                             start=(rt == 0), stop=(rt == RT - 1))
                    for h in range(2):
                        lo = cc * 1024 + h * 512
                        cb = wkC.tile([1, 512], F32, tag="cb", name="cb")
                        nc.scalar.copy(cb[:], csum[h][:])
                        nc.sync.dma_start(out=cc_in[0, lo:lo + 512], in_=cb[:])
                        nb = wkC.tile([1, 512], F32, tag="nb", name="nb")
                        nc.scalar.copy(nb[:], nsum[h][:])
                        nc.sync.dma_start(out=cc_in[1, lo:lo + 512], in_=nb[:])

            # ---------------- Stage D: combine
            with tc.tile_pool(name="workD", bufs=1) as wkD, \
                 tc.tile_pool(name="psD", bufs=2, space="PSUM") as psD:
                # collective on [colsum ; numsc]
                nc.gpsimd.collective_compute(
                    "AllReduce", Alu.add,
                    replica_groups=[list(range(NCORES))],
                    ins=[cc_in[:]], outs=[cc_out[:]])

                # mp partial: sum_i log(rowsum_i / nummp_i) over my rows
                rowsum_t = wkD.tile([128, RT], F32, tag="rowsum_t", name="rowsum_t")
                nummp_t = wkD.tile([128, RT], F32, tag="nummp_t", name="nummp_t")
                for rt in range(RT):
                    nc.vector.reduce_sum(
                        rowsum_t[:, rt:rt + 1],
                        rowsum_acc[:, rt * CC:(rt + 1) * CC],
                        axis=mybir.AxisListType.X)
                    nc.vector.reduce_sum(
                        nummp_t[:, rt:rt + 1],
                        nummp_acc[:, rt * CC:(rt + 1) * CC],
                        axis=mybir.AxisListType.X)
                recm = wkD.tile([128, RT], F32, tag="recm", name="recm")
                nc.vector.reciprocal(recm[:], nummp_t[:])
                ratm = wkD.tile([128, RT], F32, tag="ratm", name="ratm")
                nc.vector.tensor_tensor(ratm[:], rowsum_t[:], recm[:], op=Alu.mult)
                lnm = wkD.tile([128, RT], F32, tag="lnm", name="lnm")
                lsum_mp = wkD.tile([128, 1], F32, tag="lsum_mp", name="lsum_mp")
                nc.scalar.activation(lnm[:], ratm[:], Act.Ln, accum_out=lsum_mp[:])
                pmp = psD.tile([1, 1], F32, tag="pmp", name="pmp")
                nc.tensor.matmul(pmp[:], lsum_mp[:], ones_col_f32[:],
                                 start=True, stop=True)
                nc.scalar.copy(out_sb[0:1, 0:1], pmp[:])

                # sc full: sum_r log(colsum_r / numsc_r) (same on all cores)
                colsum_t = wkD.tile([128, 64], F32, tag="colsum_t", name="colsum_t")
                nc.sync.dma_start(out=colsum_t[:], in_=cc_out[0].rearrange("(p f) -> p f", p=128))
                numsc_t = wkD.tile([128, 64], F32, tag="numsc_t", name="numsc_t")
                nc.sync.dma_start(out=numsc_t[:], in_=cc_out[1].rearrange("(p f) -> p f", p=128))
                recs = wkD.tile([128, 64], F32, tag="recs", name="recs")
                nc.vector.reciprocal(recs[:], numsc_t[:])
                rats = wkD.tile([128, 64], F32, tag="rats", name="rats")
                nc.vector.tensor_tensor(rats[:], colsum_t[:], recs[:], op=Alu.mult)
                lns = wkD.tile([128, 64], F32, tag="lns", name="lns")
                lsum_sc = wkD.tile([128, 1], F32, tag="lsum_sc", name="lsum_sc")
                nc.scalar.activation(lns[:], rats[:], Act.Ln, accum_out=lsum_sc[:])
                psc = psD.tile([1, 1], F32, tag="psc", name="psc")
                nc.tensor.matmul(psc[:], lsum_sc[:], ones_col_f32[:],
                                 start=True, stop=True)
                nc.scalar.copy(out_sb[0:1, 1:2], psc[:])

                nc.sync.dma_start(out=out[:], in_=out_sb[:])

    _split_multi_waits(nc, mybir)
    return nc


def make_in_maps(z_mp, z_sc, W1, b1, W2, b2, pos):
    z_mp = np.asarray(z_mp, dtype=np.float32)
    z_sc = np.asarray(z_sc, dtype=np.float32)
    W1 = np.asarray(W1, dtype=np.float32)
    W2 = np.asarray(W2, dtype=np.float32)
    b1 = np.asarray(b1, dtype=np.float32)
    b2 = np.asarray(b2, dtype=np.float32)
    r = np.asarray(pos[0]).astype(np.int64)
    c = np.asarray(pos[1]).astype(np.int64)

    z_sct = np.ascontiguousarray(z_sc.T).astype(bf16)
    w1t = np.ascontiguousarray(W1.T).astype(bf16)
    w2t = np.ascontiguousarray(W2.T).astype(bf16)
    b1r = b1.reshape(1, HID).astype(bf16)
    b2r = b2.reshape(1, HID).astype(bf16)

    in_maps = []
    for k in range(NCORES):
        rows = slice(k * B, (k + 1) * B)
        z_mpt = np.ascontiguousarray(z_mp[rows].T).astype(bf16)

        m = np.zeros((B, N), dtype=np.float32)
        sel = (r >= k * B) & (r < (k + 1) * B)
        np.add.at(m, (r[sel] - k * B, c[sel]), 1.0)
        mask_mp = np.ascontiguousarray(
            m.reshape(RT, 128, CC, 1024).transpose(2, 0, 1, 3)).astype(bf16)

        m2 = np.zeros((B, N), dtype=np.float32)
        sel2 = (c >= k * B) & (c < (k + 1) * B)
        np.add.at(m2, (c[sel2] - k * B, r[sel2]), 1.0)
        mask_sc = np.ascontiguousarray(
            m2.reshape(RT, 128, CC, 1024).transpose(2, 0, 1, 3)).astype(bf16)

        in_maps.append({
            "z_mpt": z_mpt, "z_sct": z_sct,
            "w1t": w1t, "w2t": w2t, "b1r": b1r, "b2r": b2r,
            "mask_mp": mask_mp, "mask_sc": mask_sc,
        })
    return in_maps


def combine_outputs(results):
    mp_sum = sum(float(res["out"][0, 0]) for res in results)
    sc_sum = float(results[0]["out"][0, 1])
    loss = (LAM * mp_sum + (1.0 - LAM) * sc_sum) / N
    return np.float32(loss)


def kernel(z_mp, z_sc, W1, b1, W2, b2, pos):
    from concourse.bass_utils import run_bass_kernel_spmd
    nc = build_program()
    in_maps = make_in_maps(z_mp, z_sc, W1, b1, W2, b2, pos)
    res = run_bass_kernel_spmd(nc, in_maps, list(range(NCORES)), trace=False)
    return combine_outputs(res.results)



# revision 5
# speedup vs baseline: 4.9981x; 4.9981x over previous
"""Trainium2 Bass kernel for the contrastive loss (nn_Contrast).

loss = LAM * mean_i(-log s_mp[i]) + (1-LAM) * mean_i(-log s_sc[i])
  S = exp(cos(n1_i, n2_j)/tau);  n1 = norm(proj(z_mp)), n2 = norm(proj(z_sc))
  s_mp[i] = num_mp[i] / rowsum_i ;  s_sc[i] = num_sc[i] / colsum_i
  num_mp[i] = sum_d S[i, c_id] ;  num_sc[i] = sum_d S[c_id, i]

Wall-clock through the axon/PJRT tunnel is dominated by host->device input
bytes (~50-90 MB/s), so the kernel ships only raw shards (~2.2 MB/core):
  - z_mp / z_sc row-block shards (transposed, bf16) + W1/W2 column shards,
    AllGathered on device; every core projects the FULL z_mp and z_sc,
    normalizes, and stores row-major n1/n2 tables to DRAM (plus a
    transposed n2T in SBUF for the S row-sweep).
  - positive-pair (row, col) structure ships as a [128, 72] int32 index
    tensor; numerators are computed by indirect-DMA gathers of n1/n2 table
    rows (128 rows per descriptor batch) + per-row dot products -- no dense
    masks, and the sc numerator is row-local (no AllReduce needed for it).
  - colsum needs one ReduceScatter of [1, 8192] f32.
Host combines 8 partial scalar pairs.
"""

import numpy as np
import ml_dtypes

N = 8192
HID = 512
TAU = 0.8
LAM = 0.5
DEG = 8
NCORES = 8
B = N // NCORES          # rows per core = 1024
RT = B // 128            # 128-row tiles per core = 8
CCH = N // 512           # 512-wide col chunks = 16
KT = HID // 128          # contraction chunks = 4
WS = HID // NCORES       # weight shard cols = 64

bf16 = ml_dtypes.bfloat16


def _split_multi_waits(nc, mybir):
    """This container's walrus accepts only ONE sync-wait per instruction;
    Tile batches several. Split extras into single-wait NoOps."""
    counter = [0]
    for f in nc.m.functions:
        for bb in f.blocks:
            new_insts = []
            changed = False
            for inst in bb.instructions:
                si = inst.sync_info
                if si is not None and si.on_wait is not None and len(si.on_wait) > 1:
                    waits = list(si.on_wait)
                    for w in waits[:-1]:
                        counter[0] += 1
                        new_insts.append(mybir.InstNoOp(
                            name=f"I-wsplit-{counter[0]}",
                            engine=inst.engine,
                            sync_info=mybir.SyncInfo(on_wait=[w], on_update=[]),
                            bass_nofuse=True,
                        ))
                    inst.sync_info = mybir.SyncInfo(
                        on_wait=[waits[-1]], on_update=list(si.on_update or []))
                    changed = True
                new_insts.append(inst)
            if changed:
                bb.instructions = new_insts
    return nc


def build_program():
    import concourse.bass as bass
    import concourse.mybir as mybir
    import concourse.tile as tile
    from concourse.masks import make_identity

    dt = mybir.dt
    F32, BF16, I32 = dt.float32, dt.bfloat16, dt.int32
    Act = mybir.ActivationFunctionType
    Alu = mybir.AluOpType
    GROUPS = [list(range(NCORES))]

    nc = bass.Bass("TRN2", num_devices=NCORES)

    zmp_in = nc.dram_tensor("zmp_in", [HID, B], BF16, kind="ExternalInput")
    zsc_in = nc.dram_tensor("zsc_in", [HID, B], BF16, kind="ExternalInput")
    w_in = nc.dram_tensor("w_in", [HID, 2 * WS], BF16, kind="ExternalInput")
    bias_in = nc.dram_tensor("bias_in", [2, HID], BF16, kind="ExternalInput")
    idx_in = nc.dram_tensor("idx_in", [128, 9 * RT], I32, kind="ExternalInput")
    out = nc.dram_tensor("out", [1, 2], F32, kind="ExternalOutput")

    # collective bounce + outputs
    zmp_b = nc.dram_tensor("zmp_b", [HID, B], BF16)
    zsc_b = nc.dram_tensor("zsc_b", [HID, B], BF16)
    w_b = nc.dram_tensor("w_b", [HID, 2 * WS], BF16)
    zmp_ag = nc.dram_tensor("zmp_ag", [NCORES * HID, B], BF16, addr_space="Shared")
    zsc_ag = nc.dram_tensor("zsc_ag", [NCORES * HID, B], BF16, addr_space="Shared")
    w_ag = nc.dram_tensor("w_ag", [NCORES * HID, 2 * WS], BF16, addr_space="Shared")

    # full normalized projection tables (row-major, one sample per row)
    n1_rows = nc.dram_tensor("n1_rows", [N, HID], BF16)
    n2_rows = nc.dram_tensor("n2_rows", [N, HID], BF16)

    cs_in = nc.dram_tensor("cs_in", [1, N], F32)
    cs_rs = nc.dram_tensor("cs_rs", [1, B], F32)

    with tile.TileContext(nc) as tc:
        with tc.tile_pool(name="const", bufs=1) as constp, \
             tc.tile_pool(name="persist", bufs=1) as pers:
            # ---- collectives first: bounce + AllGather the shards
            nc.gpsimd.dma_start(out=zmp_b[:], in_=zmp_in[:])
            nc.gpsimd.dma_start(out=zsc_b[:], in_=zsc_in[:])
            nc.gpsimd.dma_start(out=w_b[:], in_=w_in[:])
            nc.gpsimd.collective_compute(
                "AllGather", Alu.bypass, replica_groups=GROUPS,
                ins=[zmp_b[:]], outs=[zmp_ag[:]])
            nc.gpsimd.collective_compute(
                "AllGather", Alu.bypass, replica_groups=GROUPS,
                ins=[zsc_b[:]], outs=[zsc_ag[:]])
            nc.gpsimd.collective_compute(
                "AllGather", Alu.bypass, replica_groups=GROUPS,
                ins=[w_b[:]], outs=[w_ag[:]])

            ones_row = constp.tile([1, B], BF16, tag="ones_row", name="ones_row")
            nc.vector.memset(ones_row[:], 1.0)
            ones_col = constp.tile([128, 1], BF16, tag="ones_col", name="ones_col")
            nc.vector.memset(ones_col[:], 1.0)
            ones_col_f32 = constp.tile([128, 1], F32, tag="ones_col_f32",
                                       name="ones_col_f32")
            nc.vector.memset(ones_col_f32[:], 1.0)
            ident = constp.tile([128, 128], BF16, tag="ident", name="ident")
            make_identity(nc, ident[:])

            # weights [128 c, 512 h_global] tiles assembled from the shards
            w1s = [constp.tile([128, HID], BF16, tag=f"w1_{k}", name=f"w1_{k}")
                   for k in range(KT)]
            w2s = [constp.tile([128, HID], BF16, tag=f"w2_{k}", name=f"w2_{k}")
                   for k in range(KT)]
            for k in range(KT):
                for g in range(NCORES):
                    rs = slice(g * HID + k * 128, g * HID + (k + 1) * 128)
                    nc.sync.dma_start(out=w1s[k][:, g * WS:(g + 1) * WS],
                                      in_=w_ag[rs, 0:WS])
                    nc.sync.dma_start(out=w2s[k][:, g * WS:(g + 1) * WS],
                                      in_=w_ag[rs, WS:2 * WS])
            b1s = constp.tile([1, HID], BF16, tag="b1s", name="b1s")
            nc.sync.dma_start(out=b1s[:], in_=bias_in[0:1, :])
            b2s = constp.tile([1, HID], BF16, tag="b2s", name="b2s")
            nc.sync.dma_start(out=b2s[:], in_=bias_in[1:2, :])
            idx_sb = constp.tile([128, 9 * RT], I32, tag="idx_sb", name="idx_sb")
            nc.sync.dma_start(out=idx_sb[:], in_=idx_in[:])

            # persistent SBUF results
            n2T = [pers.tile([128, N], BF16, tag=f"n2T_{k}", name=f"n2T_{k}")
                   for k in range(KT)]
            n1T_loc = [pers.tile([128, B], BF16, tag=f"n1Tl_{k}", name=f"n1Tl_{k}")
                       for k in range(KT)]
            n1r_loc = [pers.tile([128, HID], BF16, tag=f"n1r_{t}", name=f"n1r_{t}")
                       for t in range(RT)]
            n2r_loc = [pers.tile([128, HID], BF16, tag=f"n2r_{t}", name=f"n2r_{t}")
                       for t in range(RT)]
            rowsum_acc = pers.tile([128, RT * CCH], F32, tag="rowsum_acc",
                                   name="rowsum_acc")
            num_mp = pers.tile([128, RT], F32, tag="num_mp", name="num_mp")
            num_sc = pers.tile([128, RT], F32, tag="num_sc", name="num_sc")
            out_sb = pers.tile([1, 2], F32, tag="out_sb", name="out_sb")

            # ---------------- Stage P: full projections -> n1_rows/n2_rows
            # (+ n2T built during the z_sc pass)
            for src, z_ag, rows_dram, do_t in ((0, zmp_ag, n1_rows, False),
                                               (1, zsc_ag, n2_rows, True)):
                with tc.tile_pool(name=f"zc{src}", bufs=2) as zcp, \
                     tc.tile_pool(name=f"h1{src}", bufs=2) as h1p, \
                     tc.tile_pool(name=f"wk{src}", bufs=3) as wkp, \
                     tc.tile_pool(name=f"ps1{src}", bufs=2, space="PSUM") as ps1p, \
                     tc.tile_pool(name=f"ps2{src}", bufs=2, space="PSUM") as ps2p, \
                     tc.tile_pool(name=f"pst{src}", bufs=2, space="PSUM") as pstp:
                    for g in range(NCORES):
                        zc = [zcp.tile([128, B], BF16, tag=f"zc_{k}", name=f"zc_{k}")
                              for k in range(KT)]
                        for k in range(KT):
                            nc.sync.dma_start(
                                out=zc[k][:],
                                in_=z_ag[g * HID + k * 128:g * HID + (k + 1) * 128, :])
                        h1c = [h1p.tile([128, B], BF16, tag=f"h1_{k}", name=f"h1_{k}")
                               for k in range(KT)]
                        for ht in range(KT):
                            hsl = slice(ht * 128, (ht + 1) * 128)
                            ps = ps1p.tile([128, B], F32, tag="ps1", name="ps1")
                            for h in range(B // 512):
                                sl = slice(h * 512, (h + 1) * 512)
                                for k in range(KT):
                                    nc.tensor.matmul(ps[:, sl], w1s[k][:, hsl],
                                                     zc[k][:, sl],
                                                     start=(k == 0), stop=False)
                                nc.tensor.matmul(ps[:, sl], b1s[0:1, hsl],
                                                 ones_row[0:1, 0:512],
                                                 start=False, stop=True)
                            tmin = wkp.tile([128, B], BF16, tag="tmin", name="tmin")
                            nc.vector.tensor_scalar_min(tmin[:], ps[:], 0.0)
                            texp = wkp.tile([128, B], BF16, tag="texp", name="texp")
                            nc.scalar.activation(texp[:], tmin[:], Act.Exp)
                            nc.vector.scalar_tensor_tensor(
                                h1c[ht][:], texp[:], -1.0, ps[:],
                                op0=Alu.add, op1=Alu.max)
                        for st in range(RT):
                            ssl = slice(st * 128, (st + 1) * 128)
                            gst = g * RT + st
                            ps2 = ps2p.tile([128, HID], F32, tag="ps2", name="ps2")
                            for k in range(KT):
                                nc.tensor.matmul(ps2[:], h1c[k][:, ssl], w2s[k][:],
                                                 start=(k == 0), stop=False)
                            nc.tensor.matmul(ps2[:], ones_row[0:1, 0:128], b2s[:],
                                             start=False, stop=True)
                            sq = wkp.tile([128, HID], BF16, tag="sq", name="sq")
                            nrm2 = wkp.tile([128, 1], F32, tag="nrm2", name="nrm2")
                            nc.scalar.activation(sq[:], ps2[:], Act.Square,
                                                 accum_out=nrm2[:])
                            nrm = wkp.tile([128, 1], F32, tag="nrm", name="nrm")
                            nc.scalar.activation(nrm[:], nrm2[:], Act.Sqrt)
                            rinv = wkp.tile([128, 1], F32, tag="rinv", name="rinv")
                            nc.vector.reciprocal(rinv[:], nrm[:])
                            nrows = wkp.tile([128, HID], BF16, tag="nrows",
                                             name="nrows")
                            nc.scalar.activation(nrows[:], ps2[:], Act.Copy,
                                                 scale=rinv[:, 0:1])
                            nc.sync.dma_start(
                                out=rows_dram[gst * 128:(gst + 1) * 128, :],
                                in_=nrows[:])
                            if do_t:
                                for hb in range(KT):
                                    pst = pstp.tile([128, 128], BF16, tag="pst",
                                                    name="pst")
                                    nc.tensor.transpose(
                                        pst[:], nrows[:, hb * 128:(hb + 1) * 128],
                                        ident[:])
                                    nc.vector.tensor_copy(
                                        n2T[hb][:, gst * 128:(gst + 1) * 128],
                                        pst[:])

            # ---------------- Stage L: local rows via gather + transpose
            with tc.tile_pool(name="pstL", bufs=2, space="PSUM") as pstL:
                for t in range(RT):
                    nc.gpsimd.indirect_dma_start(
                        out=n1r_loc[t][:], out_offset=None, in_=n1_rows[:],
                        in_offset=bass.IndirectOffsetOnAxis(
                            ap=idx_sb[:, 8 * RT + t:8 * RT + t + 1], axis=0))
                    nc.gpsimd.indirect_dma_start(
                        out=n2r_loc[t][:], out_offset=None, in_=n2_rows[:],
                        in_offset=bass.IndirectOffsetOnAxis(
                            ap=idx_sb[:, 8 * RT + t:8 * RT + t + 1], axis=0))
                    for hb in range(KT):
                        pst = pstL.tile([128, 128], BF16, tag="pstL", name="pstL")
                        nc.tensor.transpose(
                            pst[:], n1r_loc[t][:, hb * 128:(hb + 1) * 128], ident[:])
                        nc.vector.tensor_copy(
                            n1T_loc[hb][:, t * 128:(t + 1) * 128], pst[:])

            # ---------------- Stage S: S row-sweep -> rowsum + colsum partials
            with tc.tile_pool(name="wkS", bufs=3) as wkS, \
                 tc.tile_pool(name="psS", bufs=3, space="PSUM") as psS, \
                 tc.tile_pool(name="psSc", bufs=2, space="PSUM") as psSc:
                for cc in range(CCH):
                    csl = slice(cc * 512, (cc + 1) * 512)
                    csum = psSc.tile([1, 512], F32, tag="csum", name="csum")
                    for rt in range(RT):
                        rsl = slice(rt * 128, (rt + 1) * 128)
                        sp = psS.tile([128, 512], F32, tag="spS", name="spS")
                        for k in range(KT):
                            nc.tensor.matmul(sp[:], n1T_loc[k][:, rsl],
                                             n2T[k][:, csl],
                                             start=(k == 0), stop=(k == KT - 1))
                        s_sb = wkS.tile([128, 512], BF16, tag="s_sb", name="s_sb")
                        idx = rt * CCH + cc
                        nc.scalar.activation(
                            s_sb[:], sp[:], Act.Exp, scale=1.0 / TAU,
                            accum_out=rowsum_acc[:, idx:idx + 1])
                        nc.tensor.matmul(csum[:], ones_col[:], s_sb[:],
                                         start=(rt == 0), stop=(rt == RT - 1))
                    cb = wkS.tile([1, 512], F32, tag="cb", name="cb")
                    nc.scalar.copy(cb[:], csum[:])
                    nc.sync.dma_start(out=cs_in[0, csl], in_=cb[:])

            # ---------------- Stage G: edge gathers -> numerators
            with tc.tile_pool(name="wkG", bufs=4) as wkG:
                for t in range(RT):
                    dots_mp = wkG.tile([128, DEG], F32, tag="dots_mp",
                                       name="dots_mp")
                    dots_sc = wkG.tile([128, DEG], F32, tag="dots_sc",
                                       name="dots_sc")
                    for d in range(DEG):
                        j = t * DEG + d
                        g2 = wkG.tile([128, HID], BF16, tag="g2", name="g2")
                        nc.gpsimd.indirect_dma_start(
                            out=g2[:], out_offset=None, in_=n2_rows[:],
                            in_offset=bass.IndirectOffsetOnAxis(
                                ap=idx_sb[:, j:j + 1], axis=0))
                        dis = wkG.tile([128, HID], BF16, tag="dis", name="dis")
                        nc.vector.scalar_tensor_tensor(
                            dis[:], g2[:], 1.0, n1r_loc[t][:],
                            op0=Alu.mult, op1=Alu.mult,
                            accum_out=dots_mp[:, d:d + 1])
                        g1 = wkG.tile([128, HID], BF16, tag="g1", name="g1")
                        nc.gpsimd.indirect_dma_start(
                            out=g1[:], out_offset=None, in_=n1_rows[:],
                            in_offset=bass.IndirectOffsetOnAxis(
                                ap=idx_sb[:, j:j + 1], axis=0))
                        dis2 = wkG.tile([128, HID], BF16, tag="dis2", name="dis2")
                        nc.vector.scalar_tensor_tensor(
                            dis2[:], g1[:], 1.0, n2r_loc[t][:],
                            op0=Alu.mult, op1=Alu.mult,
                            accum_out=dots_sc[:, d:d + 1])
                    e_mp = wkG.tile([128, DEG], F32, tag="e_mp", name="e_mp")
                    nc.scalar.activation(e_mp[:], dots_mp[:], Act.Exp,
                                         scale=1.0 / TAU)
                    nc.vector.reduce_sum(num_mp[:, t:t + 1], e_mp[:],
                                         axis=mybir.AxisListType.X)
                    e_sc = wkG.tile([128, DEG], F32, tag="e_sc", name="e_sc")
                    nc.scalar.activation(e_sc[:], dots_sc[:], Act.Exp,
                                         scale=1.0 / TAU)
                    nc.vector.reduce_sum(num_sc[:, t:t + 1], e_sc[:],
                                         axis=mybir.AxisListType.X)

            # ---------------- Stage D: combine
            with tc.tile_pool(name="wkD", bufs=1) as wkD, \
                 tc.tile_pool(name="psD", bufs=2, space="PSUM") as psD:
                nc.gpsimd.collective_compute(
                    "ReduceScatter", Alu.add, replica_groups=GROUPS,
                    ins=[cs_in[:]], outs=[cs_rs[:]])

                # mp partial: sum_i log(rowsum_i / num_mp_i) over my rows
                rs_t = wkD.tile([128, RT], F32, tag="rs_t", name="rs_t")
                for rt in range(RT):
                    nc.vector.reduce_sum(
                        rs_t[:, rt:rt + 1],
                        rowsum_acc[:, rt * CCH:(rt + 1) * CCH],
                        axis=mybir.AxisListType.X)
                recm = wkD.tile([128, RT], F32, tag="recm", name="recm")
                nc.vector.reciprocal(recm[:], num_mp[:])
                ratm = wkD.tile([128, RT], F32, tag="ratm", name="ratm")
                nc.vector.tensor_tensor(ratm[:], rs_t[:], recm[:], op=Alu.mult)
                lnm = wkD.tile([128, RT], F32, tag="lnm", name="lnm")
                lsum_mp = wkD.tile([128, 1], F32, tag="lsum_mp", name="lsum_mp")
                nc.scalar.activation(lnm[:], ratm[:], Act.Ln, accum_out=lsum_mp[:])
                pmp = psD.tile([1, 1], F32, tag="pmp", name="pmp")
                nc.tensor.matmul(pmp[:], lsum_mp[:], ones_col_f32[:],
                                 start=True, stop=True)
                nc.scalar.copy(out_sb[0:1, 0:1], pmp[:])

                # sc partial: sum_i log(colsum_i / num_sc_i) over my rows
                colsum_loc = wkD.tile([128, RT], F32, tag="colsum_loc",
                                      name="colsum_loc")
                nc.sync.dma_start(
                    out=colsum_loc[:],
                    in_=cs_rs[0].rearrange("(f p) -> p f", p=128))
                recs = wkD.tile([128, RT], F32, tag="recs", name="recs")
                nc.vector.reciprocal(recs[:], num_sc[:])
                rats = wkD.tile([128, RT], F32, tag="rats", name="rats")
                nc.vector.tensor_tensor(rats[:], colsum_loc[:], recs[:],
                                        op=Alu.mult)
                lns = wkD.tile([128, RT], F32, tag="lns", name="lns")
                lsum_sc = wkD.tile([128, 1], F32, tag="lsum_sc", name="lsum_sc")
                nc.scalar.activation(lns[:], rats[:], Act.Ln, accum_out=lsum_sc[:])
                psc = psD.tile([1, 1], F32, tag="psc", name="psc")
                nc.tensor.matmul(psc[:], lsum_sc[:], ones_col_f32[:],
                                 start=True, stop=True)
                nc.scalar.copy(out_sb[0:1, 1:2], psc[:])

                nc.sync.dma_start(out=out[:], in_=out_sb[:])

    _split_multi_waits(nc, mybir)
    return nc


def make_in_maps(z_mp, z_sc, W1, b1, W2, b2, pos):
    z_mp = np.asarray(z_mp, dtype=np.float32)
    z_sc = np.asarray(z_sc, dtype=np.float32)
    W1 = np.asarray(W1, dtype=np.float32)
    W2 = np.asarray(W2, dtype=np.float32)
    b1 = np.asarray(b1, dtype=np.float32)
    b2 = np.asarray(b2, dtype=np.float32)
    r = np.asarray(pos[0]).astype(np.int64)
    c = np.asarray(pos[1]).astype(np.int64)

    # per-row sorted edge columns: row i's DEG columns, any order
    order = np.argsort(r, kind="stable")
    c_by_row = c[order].reshape(N, DEG)  # requires DEG edges per row

    w1t = np.ascontiguousarray(W1.T).astype(bf16)  # [in c, out h]
    w2t = np.ascontiguousarray(W2.T).astype(bf16)
    bias = np.stack([b1, b2]).astype(bf16)  # [2, 512]

    in_maps = []
    for k in range(NCORES):
        rows = slice(k * B, (k + 1) * B)
        zmp_s = np.ascontiguousarray(z_mp[rows].T).astype(bf16)  # [512, 1024]
        zsc_s = np.ascontiguousarray(z_sc[rows].T).astype(bf16)
        w_s = np.ascontiguousarray(
            np.concatenate([w1t[:, k * WS:(k + 1) * WS],
                            w2t[:, k * WS:(k + 1) * WS]], axis=1)).astype(bf16)

        idx = np.zeros((128, 9 * RT), dtype=np.int32)
        cb = c_by_row[rows].reshape(RT, 128, DEG)  # [t, p, d]
        for t in range(RT):
            idx[:, t * DEG:(t + 1) * DEG] = cb[t]
            idx[:, 8 * RT + t] = k * B + t * 128 + np.arange(128)

        in_maps.append({
            "zmp_in": zmp_s, "zsc_in": zsc_s, "w_in": w_s,
            "bias_in": bias, "idx_in": idx,
        })
    return in_maps


def combine_outputs(results):
    mp_sum = sum(float(res["out"][0, 0]) for res in results)
    sc_sum = sum(float(res["out"][0, 1]) for res in results)
    loss = (LAM * mp_sum + (1.0 - LAM) * sc_sum) / N
    return np.float32(loss)


def kernel(z_mp, z_sc, W1, b1, W2, b2, pos):
    from concourse.bass_utils import run_bass_kernel_spmd
    nc = build_program()
    in_maps = make_in_maps(z_mp, z_sc, W1, b1, W2, b2, pos)
    res = run_bass_kernel_spmd(nc, in_maps, list(range(NCORES)), trace=False)
    return combine_outputs(res.results)


# revision 12
# speedup vs baseline: 6.1402x; 1.2285x over previous
"""Trainium2 Bass kernel for the contrastive loss (nn_Contrast).

loss = LAM * mean_i(-log s_mp[i]) + (1-LAM) * mean_i(-log s_sc[i])
  S = exp(cos(n1_i, n2_j)/tau);  n1 = norm(proj(z_mp)), n2 = norm(proj(z_sc))
  s_mp[i] = num_mp[i] / rowsum_i ;  s_sc[i] = num_sc[i] / colsum_i
  num_mp[i] = sum_d S[i, c_id] ;  num_sc[i] = sum_d S[c_id, i]

Wall-clock through the axon/PJRT tunnel is dominated by host->device input
bytes (~50-90 MB/s), so the kernel ships only raw shards (~2.2 MB/core):
  - z_mp / z_sc row-block shards (transposed, bf16) + W1/W2 column shards,
    AllGathered on device; every core projects the FULL z_mp and z_sc,
    normalizes, and stores row-major n1/n2 tables to DRAM (plus a
    transposed n2T in SBUF for the S row-sweep).
  - positive-pair (row, col) structure ships as a [128, 72] int32 index
    tensor; numerators are computed by indirect-DMA gathers of n1/n2 table
    rows (128 rows per descriptor batch) + per-row dot products -- no dense
    masks, and the sc numerator is row-local (no AllReduce needed for it).
  - colsum needs one ReduceScatter of [1, 8192] f32.
Host combines 8 partial scalar pairs.
"""

import numpy as np
import ml_dtypes

N = 8192
HID = 512
TAU = 0.8
LAM = 0.5
DEG = 8
NCORES = 8
B = N // NCORES          # rows per core = 1024
RT = B // 128            # 128-row tiles per core = 8
CCH = N // 512           # 512-wide col chunks = 16
KT = HID // 128          # contraction chunks = 4
WS = HID // NCORES       # weight shard cols = 64

bf16 = ml_dtypes.bfloat16

# flat element offsets inside the packed per-core input (all bf16)
OFF_ZMP = 0                      # z_mp shard, transposed [HID, B]
OFF_ZSC = OFF_ZMP + HID * B      # z_sc shard, transposed [HID, B]
OFF_W1 = OFF_ZSC + HID * B       # W1.T column shard [HID, WS]
OFF_W2 = OFF_W1 + HID * WS       # W2.T column shard [HID, WS]
OFF_B1 = OFF_W2 + HID * WS       # b1 [HID]
OFF_B2 = OFF_B1 + HID            # b2 [HID]
OFF_IDX = OFF_B2 + HID           # indices as bf16 (hi|lo) [128, 2*9*RT]
TOT = OFF_IDX + 128 * 2 * 9 * RT


def _split_multi_waits(nc, mybir):
    """This container's walrus accepts only ONE sync-wait per instruction;
    Tile batches several. Split extras into single-wait NoOps."""
    counter = [0]
    for f in nc.m.functions:
        for bb in f.blocks:
            new_insts = []
            changed = False
            for inst in bb.instructions:
                si = inst.sync_info
                if si is not None and si.on_wait is not None and len(si.on_wait) > 1:
                    waits = list(si.on_wait)
                    for w in waits[:-1]:
                        counter[0] += 1
                        new_insts.append(mybir.InstNoOp(
                            name=f"I-wsplit-{counter[0]}",
                            engine=inst.engine,
                            sync_info=mybir.SyncInfo(on_wait=[w], on_update=[]),
                            bass_nofuse=True,
                        ))
                    inst.sync_info = mybir.SyncInfo(
                        on_wait=[waits[-1]], on_update=list(si.on_update or []))
                    changed = True
                new_insts.append(inst)
            if changed:
                bb.instructions = new_insts
    return nc


def build_program():
    import concourse.bass as bass
    import concourse.mybir as mybir
    import concourse.tile as tile
    from concourse.masks import make_identity

    dt = mybir.dt
    F32, BF16, I32 = dt.float32, dt.bfloat16, dt.int32
    Act = mybir.ActivationFunctionType
    Alu = mybir.AluOpType
    GROUPS = [list(range(NCORES))]

    nc = bass.Bass("TRN2", num_devices=NCORES)

    mega_in = nc.dram_tensor("mega_in", [TOT], BF16, kind="ExternalInput")
    out = nc.dram_tensor("out", [1, 2], F32, kind="ExternalOutput")

    # collective bounce + output
    mega_b = nc.dram_tensor("mega_b", [TOT], BF16)
    mega_ag = nc.dram_tensor("mega_ag", [NCORES * TOT], BF16, addr_space="Shared")

    # full normalized projection tables (row-major, one sample per row)
    n1_rows = nc.dram_tensor("n1_rows", [N, HID], BF16)
    n2_rows = nc.dram_tensor("n2_rows", [N, HID], BF16)

    cs_in = nc.dram_tensor("cs_in", [1, N], F32)
    cs_rs = nc.dram_tensor("cs_rs", [1, B], F32)

    with tile.TileContext(nc) as tc:
        with tc.tile_pool(name="const", bufs=1) as constp, \
             tc.tile_pool(name="persist", bufs=1) as pers:
            # ---- collective first: bounce + AllGather the packed shard
            nc.gpsimd.dma_start(
                out=mega_b[:].rearrange("(p f) -> p f", p=128),
                in_=mega_in[:].rearrange("(p f) -> p f", p=128))
            nc.gpsimd.collective_compute(
                "AllGather", Alu.bypass, replica_groups=GROUPS,
                ins=[mega_b[:].rearrange("(p f) -> p f", p=128)],
                outs=[mega_ag[:].rearrange("(p f) -> p f", p=1024)])

            ones_row = constp.tile([1, B], BF16, tag="ones_row", name="ones_row")
            nc.vector.memset(ones_row[:], 1.0)
            ones_col = constp.tile([128, 1], BF16, tag="ones_col", name="ones_col")
            nc.vector.memset(ones_col[:], 1.0)
            ones_col_f32 = constp.tile([128, 1], F32, tag="ones_col_f32",
                                       name="ones_col_f32")
            nc.vector.memset(ones_col_f32[:], 1.0)
            ident = constp.tile([128, 128], BF16, tag="ident", name="ident")
            make_identity(nc, ident[:])

            # weights [128 c, 512 h_global] tiles assembled from the shards
            w1s = [constp.tile([128, HID], BF16, tag=f"w1_{k}", name=f"w1_{k}")
                   for k in range(KT)]
            w2s = [constp.tile([128, HID], BF16, tag=f"w2_{k}", name=f"w2_{k}")
                   for k in range(KT)]
            for k in range(KT):
                for g in range(NCORES):
                    a1 = g * TOT + OFF_W1 + k * 128 * WS
                    a2 = g * TOT + OFF_W2 + k * 128 * WS
                    nc.sync.dma_start(
                        out=w1s[k][:, g * WS:(g + 1) * WS],
                        in_=mega_ag[a1:a1 + 128 * WS].rearrange(
                            "(p f) -> p f", p=128))
                    nc.sync.dma_start(
                        out=w2s[k][:, g * WS:(g + 1) * WS],
                        in_=mega_ag[a2:a2 + 128 * WS].rearrange(
                            "(p f) -> p f", p=128))
            b1s = constp.tile([1, HID], BF16, tag="b1s", name="b1s")
            nc.sync.dma_start(out=b1s[:], in_=mega_in[OFF_B1:OFF_B1 + HID]
                              .rearrange("(p f) -> p f", p=1))
            b2s = constp.tile([1, HID], BF16, tag="b2s", name="b2s")
            nc.sync.dma_start(out=b2s[:], in_=mega_in[OFF_B2:OFF_B2 + HID]
                              .rearrange("(p f) -> p f", p=1))
            # indices ship as exact bf16 (hi, lo) pairs: idx = hi*64 + lo
            idxhl = constp.tile([128, 2 * 9 * RT], BF16, tag="idxhl",
                                name="idxhl")
            nc.sync.dma_start(
                out=idxhl[:],
                in_=mega_in[OFF_IDX:OFF_IDX + 128 * 2 * 9 * RT].rearrange(
                    "(p f) -> p f", p=128))
            idx_sb = constp.tile([128, 9 * RT], I32, tag="idx_sb", name="idx_sb")
            nc.vector.scalar_tensor_tensor(
                idx_sb[:], idxhl[:, 0:9 * RT], 64.0, idxhl[:, 9 * RT:2 * 9 * RT],
                op0=Alu.mult, op1=Alu.add)

            # persistent SBUF results
            n2T = [pers.tile([128, N], BF16, tag=f"n2T_{k}", name=f"n2T_{k}")
                   for k in range(KT)]
            n1T_loc = [pers.tile([128, B], BF16, tag=f"n1Tl_{k}", name=f"n1Tl_{k}")
                       for k in range(KT)]
            n1r_loc = [pers.tile([128, HID], BF16, tag=f"n1r_{t}", name=f"n1r_{t}")
                       for t in range(RT)]
            n2r_loc = [pers.tile([128, HID], BF16, tag=f"n2r_{t}", name=f"n2r_{t}")
                       for t in range(RT)]
            rowsum_acc = pers.tile([128, RT * CCH], F32, tag="rowsum_acc",
                                   name="rowsum_acc")
            num_mp = pers.tile([128, RT], F32, tag="num_mp", name="num_mp")
            num_sc = pers.tile([128, RT], F32, tag="num_sc", name="num_sc")
            out_sb = pers.tile([1, 2], F32, tag="out_sb", name="out_sb")

            # ---------------- Stage P: full projections -> n1_rows/n2_rows
            # (+ n2T built during the z_sc pass)
            for src, z_off, rows_dram, do_t in ((0, OFF_ZMP, n1_rows, False),
                                                (1, OFF_ZSC, n2_rows, True)):
                with tc.tile_pool(name=f"zc{src}", bufs=2) as zcp, \
                     tc.tile_pool(name=f"h1{src}", bufs=2) as h1p, \
                     tc.tile_pool(name=f"wk{src}", bufs=3) as wkp, \
                     tc.tile_pool(name=f"ps1{src}", bufs=2, space="PSUM") as ps1p, \
                     tc.tile_pool(name=f"ps2{src}", bufs=2, space="PSUM") as ps2p, \
                     tc.tile_pool(name=f"pst{src}", bufs=2, space="PSUM") as pstp:
                    for g in range(NCORES):
                        zc = [zcp.tile([128, B], BF16, tag=f"zc_{k}", name=f"zc_{k}")
                              for k in range(KT)]
                        for k in range(KT):
                            a = g * TOT + z_off + k * 128 * B
                            nc.sync.dma_start(
                                out=zc[k][:],
                                in_=mega_ag[a:a + 128 * B].rearrange(
                                    "(p f) -> p f", p=128))
                        h1c = [h1p.tile([128, B], BF16, tag=f"h1_{k}", name=f"h1_{k}")
                               for k in range(KT)]
                        for ht in range(KT):
                            hsl = slice(ht * 128, (ht + 1) * 128)
                            ps = ps1p.tile([128, B], F32, tag="ps1", name="ps1")
                            for h in range(B // 512):
                                sl = slice(h * 512, (h + 1) * 512)
                                for k in range(KT):
                                    nc.tensor.matmul(ps[:, sl], w1s[k][:, hsl],
                                                     zc[k][:, sl],
                                                     start=(k == 0), stop=False)
                                nc.tensor.matmul(ps[:, sl], b1s[0:1, hsl],
                                                 ones_row[0:1, 0:512],
                                                 start=False, stop=True)
                            tmin = wkp.tile([128, B], BF16, tag="tmin", name="tmin")
                            nc.vector.tensor_scalar_min(tmin[:], ps[:], 0.0)
                            texp = wkp.tile([128, B], BF16, tag="texp", name="texp")
                            nc.scalar.activation(texp[:], tmin[:], Act.Exp)
                            nc.vector.scalar_tensor_tensor(
                                h1c[ht][:], texp[:], -1.0, ps[:],
                                op0=Alu.add, op1=Alu.max)
                        for st in range(RT):
                            ssl = slice(st * 128, (st + 1) * 128)
                            gst = g * RT + st
                            ps2 = ps2p.tile([128, HID], F32, tag="ps2", name="ps2")
                            for k in range(KT):
                                nc.tensor.matmul(ps2[:], h1c[k][:, ssl], w2s[k][:],
                                                 start=(k == 0), stop=False)
                            nc.tensor.matmul(ps2[:], ones_row[0:1, 0:128], b2s[:],
                                             start=False, stop=True)
                            sq = wkp.tile([128, HID], BF16, tag="sq", name="sq")
                            nrm2 = wkp.tile([128, 1], F32, tag="nrm2", name="nrm2")
                            nc.scalar.activation(sq[:], ps2[:], Act.Square,
                                                 accum_out=nrm2[:])
                            nrm = wkp.tile([128, 1], F32, tag="nrm", name="nrm")
                            nc.scalar.activation(nrm[:], nrm2[:], Act.Sqrt)
                            rinv = wkp.tile([128, 1], F32, tag="rinv", name="rinv")
                            nc.vector.reciprocal(rinv[:], nrm[:])
                            nrows = wkp.tile([128, HID], BF16, tag="nrows",
                                             name="nrows")
                            nc.scalar.activation(nrows[:], ps2[:], Act.Copy,
                                                 scale=rinv[:, 0:1])
                            nc.sync.dma_start(
                                out=rows_dram[gst * 128:(gst + 1) * 128, :],
                                in_=nrows[:])
                            if do_t:
                                for hb in range(KT):
                                    pst = pstp.tile([128, 128], BF16, tag="pst",
                                                    name="pst")
                                    nc.tensor.transpose(
                                        pst[:], nrows[:, hb * 128:(hb + 1) * 128],
                                        ident[:])
                                    nc.vector.tensor_copy(
                                        n2T[hb][:, gst * 128:(gst + 1) * 128],
                                        pst[:])

            # ---------------- Stage L: local rows via gather + transpose
            with tc.tile_pool(name="pstL", bufs=2, space="PSUM") as pstL:
                for t in range(RT):
                    nc.gpsimd.indirect_dma_start(
                        out=n1r_loc[t][:], out_offset=None, in_=n1_rows[:],
                        in_offset=bass.IndirectOffsetOnAxis(
                            ap=idx_sb[:, 8 * RT + t:8 * RT + t + 1], axis=0))
                    nc.gpsimd.indirect_dma_start(
                        out=n2r_loc[t][:], out_offset=None, in_=n2_rows[:],
                        in_offset=bass.IndirectOffsetOnAxis(
                            ap=idx_sb[:, 8 * RT + t:8 * RT + t + 1], axis=0))
                    for hb in range(KT):
                        pst = pstL.tile([128, 128], BF16, tag="pstL", name="pstL")
                        nc.tensor.transpose(
                            pst[:], n1r_loc[t][:, hb * 128:(hb + 1) * 128], ident[:])
                        nc.vector.tensor_copy(
                            n1T_loc[hb][:, t * 128:(t + 1) * 128], pst[:])

            # ---------------- Stage S: S row-sweep -> rowsum + colsum partials
            with tc.tile_pool(name="wkS", bufs=3) as wkS, \
                 tc.tile_pool(name="psS", bufs=3, space="PSUM") as psS, \
                 tc.tile_pool(name="psSc", bufs=2, space="PSUM") as psSc:
                for cc in range(CCH):
                    csl = slice(cc * 512, (cc + 1) * 512)
                    csum = psSc.tile([1, 512], F32, tag="csum", name="csum")
                    for rt in range(RT):
                        rsl = slice(rt * 128, (rt + 1) * 128)
                        sp = psS.tile([128, 512], F32, tag="spS", name="spS")
                        for k in range(KT):
                            nc.tensor.matmul(sp[:], n1T_loc[k][:, rsl],
                                             n2T[k][:, csl],
                                             start=(k == 0), stop=(k == KT - 1))
                        s_sb = wkS.tile([128, 512], BF16, tag="s_sb", name="s_sb")
                        idx = rt * CCH + cc
                        nc.scalar.activation(
                            s_sb[:], sp[:], Act.Exp, scale=1.0 / TAU,
                            accum_out=rowsum_acc[:, idx:idx + 1])
                        nc.tensor.matmul(csum[:], ones_col[:], s_sb[:],
                                         start=(rt == 0), stop=(rt == RT - 1))
                    cb = wkS.tile([1, 512], F32, tag="cb", name="cb")
                    nc.scalar.copy(cb[:], csum[:])
                    nc.sync.dma_start(out=cs_in[0, csl], in_=cb[:])

            # ---------------- Stage G: edge gathers -> numerators
            with tc.tile_pool(name="wkG", bufs=4) as wkG:
                for t in range(RT):
                    dots_mp = wkG.tile([128, DEG], F32, tag="dots_mp",
                                       name="dots_mp")
                    dots_sc = wkG.tile([128, DEG], F32, tag="dots_sc",
                                       name="dots_sc")
                    for d in range(DEG):
                        j = t * DEG + d
                        g2 = wkG.tile([128, HID], BF16, tag="g2", name="g2")
                        nc.gpsimd.indirect_dma_start(
                            out=g2[:], out_offset=None, in_=n2_rows[:],
                            in_offset=bass.IndirectOffsetOnAxis(
                                ap=idx_sb[:, j:j + 1], axis=0))
                        dis = wkG.tile([128, HID], BF16, tag="dis", name="dis")
                        nc.vector.scalar_tensor_tensor(
                            dis[:], g2[:], 1.0, n1r_loc[t][:],
                            op0=Alu.mult, op1=Alu.mult,
                            accum_out=dots_mp[:, d:d + 1])
                        g1 = wkG.tile([128, HID], BF16, tag="g1", name="g1")
                        nc.gpsimd.indirect_dma_start(
                            out=g1[:], out_offset=None, in_=n1_rows[:],
                            in_offset=bass.IndirectOffsetOnAxis(
                                ap=idx_sb[:, j:j + 1], axis=0))
                        dis2 = wkG.tile([128, HID], BF16, tag="dis2", name="dis2")
                        nc.vector.scalar_tensor_tensor(
                            dis2[:], g1[:], 1.0, n2r_loc[t][:],
                            op0=Alu.mult, op1=Alu.mult,
                            accum_out=dots_sc[:, d:d + 1])
                    e_mp = wkG.tile([128, DEG], F32, tag="e_mp", name="e_mp")
                    nc.scalar.activation(e_mp[:], dots_mp[:], Act.Exp,
                                         scale=1.0 / TAU)
                    nc.vector.reduce_sum(num_mp[:, t:t + 1], e_mp[:],
                                         axis=mybir.AxisListType.X)
                    e_sc = wkG.tile([128, DEG], F32, tag="e_sc", name="e_sc")
                    nc.scalar.activation(e_sc[:], dots_sc[:], Act.Exp,
                                         scale=1.0 / TAU)
                    nc.vector.reduce_sum(num_sc[:, t:t + 1], e_sc[:],
                                         axis=mybir.AxisListType.X)

            # ---------------- Stage D: combine
            with tc.tile_pool(name="wkD", bufs=1) as wkD, \
                 tc.tile_pool(name="psD", bufs=2, space="PSUM") as psD:
                nc.gpsimd.collective_compute(
                    "ReduceScatter", Alu.add, replica_groups=GROUPS,
                    ins=[cs_in[:]], outs=[cs_rs[:]])

                # mp partial: sum_i log(rowsum_i / num_mp_i) over my rows
                rs_t = wkD.tile([128, RT], F32, tag="rs_t", name="rs_t")
                for rt in range(RT):
                    nc.vector.reduce_sum(
                        rs_t[:, rt:rt + 1],
                        rowsum_acc[:, rt * CCH:(rt + 1) * CCH],
                        axis=mybir.AxisListType.X)
                recm = wkD.tile([128, RT], F32, tag="recm", name="recm")
                nc.vector.reciprocal(recm[:], num_mp[:])
                ratm = wkD.tile([128, RT], F32, tag="ratm", name="ratm")
                nc.vector.tensor_tensor(ratm[:], rs_t[:], recm[:], op=Alu.mult)
                lnm = wkD.tile([128, RT], F32, tag="lnm", name="lnm")
                lsum_mp = wkD.tile([128, 1], F32, tag="lsum_mp", name="lsum_mp")
                nc.scalar.activation(lnm[:], ratm[:], Act.Ln, accum_out=lsum_mp[:])
                pmp = psD.tile([1, 1], F32, tag="pmp", name="pmp")
                nc.tensor.matmul(pmp[:], lsum_mp[:], ones_col_f32[:],
                                 start=True, stop=True)
                nc.scalar.copy(out_sb[0:1, 0:1], pmp[:])

                # sc partial: sum_i log(colsum_i / num_sc_i) over my rows
                colsum_loc = wkD.tile([128, RT], F32, tag="colsum_loc",
                                      name="colsum_loc")
                nc.sync.dma_start(
                    out=colsum_loc[:],
                    in_=cs_rs[0].rearrange("(f p) -> p f", p=128))
                recs = wkD.tile([128, RT], F32, tag="recs", name="recs")
                nc.vector.reciprocal(recs[:], num_sc[:])
                rats = wkD.tile([128, RT], F32, tag="rats", name="rats")
                nc.vector.tensor_tensor(rats[:], colsum_loc[:], recs[:],
                                        op=Alu.mult)
                lns = wkD.tile([128, RT], F32, tag="lns", name="lns")
                lsum_sc = wkD.tile([128, 1], F32, tag="lsum_sc", name="lsum_sc")
                nc.scalar.activation(lns[:], rats[:], Act.Ln, accum_out=lsum_sc[:])
                psc = psD.tile([1, 1], F32, tag="psc", name="psc")
                nc.tensor.matmul(psc[:], lsum_sc[:], ones_col_f32[:],
                                 start=True, stop=True)
                nc.scalar.copy(out_sb[0:1, 1:2], psc[:])

                nc.sync.dma_start(out=out[:], in_=out_sb[:])

    _split_multi_waits(nc, mybir)
    return nc


def make_in_maps(z_mp, z_sc, W1, b1, W2, b2, pos):
    z_mp = np.asarray(z_mp, dtype=np.float32)
    z_sc = np.asarray(z_sc, dtype=np.float32)
    W1 = np.asarray(W1, dtype=np.float32)
    W2 = np.asarray(W2, dtype=np.float32)
    b1 = np.asarray(b1, dtype=np.float32)
    b2 = np.asarray(b2, dtype=np.float32)
    r = np.asarray(pos[0]).astype(np.int64)
    c = np.asarray(pos[1]).astype(np.int64)

    # per-row sorted edge columns: row i's DEG columns, any order
    order = np.argsort(r, kind="stable")
    c_by_row = c[order].reshape(N, DEG)  # requires DEG edges per row

    w1t = np.ascontiguousarray(W1.T).astype(bf16)  # [in c, out h]
    w2t = np.ascontiguousarray(W2.T).astype(bf16)

    in_maps = []
    for k in range(NCORES):
        rows = slice(k * B, (k + 1) * B)

        idx = np.zeros((128, 9 * RT), dtype=np.int32)
        cb = c_by_row[rows].reshape(RT, 128, DEG)  # [t, p, d]
        for t in range(RT):
            idx[:, t * DEG:(t + 1) * DEG] = cb[t]
            idx[:, 8 * RT + t] = k * B + t * 128 + np.arange(128)

        mega = np.empty(TOT, dtype=bf16)
        mega[OFF_ZMP:OFF_ZSC] = z_mp[rows].T.astype(bf16).ravel()
        mega[OFF_ZSC:OFF_W1] = z_sc[rows].T.astype(bf16).ravel()
        mega[OFF_W1:OFF_W2] = w1t[:, k * WS:(k + 1) * WS].ravel()
        mega[OFF_W2:OFF_B1] = w2t[:, k * WS:(k + 1) * WS].ravel()
        mega[OFF_B1:OFF_B2] = b1.astype(bf16)
        mega[OFF_B2:OFF_IDX] = b2.astype(bf16)
        hilo = np.empty((128, 2 * 9 * RT), dtype=bf16)
        hilo[:, 0:9 * RT] = (idx // 64).astype(bf16)
        hilo[:, 9 * RT:] = (idx % 64).astype(bf16)
        mega[OFF_IDX:TOT] = hilo.ravel()

        in_maps.append({"mega_in": mega})
    return in_maps


def combine_outputs(results):
    mp_sum = sum(float(res["out"][0, 0]) for res in results)
    sc_sum = sum(float(res["out"][0, 1]) for res in results)
    loss = (LAM * mp_sum + (1.0 - LAM) * sc_sum) / N
    return np.float32(loss)


def kernel(z_mp, z_sc, W1, b1, W2, b2, pos):
    from concourse.bass_utils import run_bass_kernel_spmd
    nc = build_program()
    in_maps = make_in_maps(z_mp, z_sc, W1, b1, W2, b2, pos)
    res = run_bass_kernel_spmd(nc, in_maps, list(range(NCORES)), trace=False)
    return combine_outputs(res.results)
